# revision 18
# baseline (speedup 1.0000x reference)
"""Trainium2 Bass kernel for nn_Decoder (2-layer LSTM decoder + Luong attention
+ vocab projection + log-softmax), 8-way batch-parallel SPMD.

Sharding: data-parallel over batch (B=32 -> 4 per core). The LSTM recurrence,
attention, projection and log-softmax for a core's 4 batch elements are fully
local, so no collectives are needed. Each core's projection covers all
T*B_local = 256 rows x full vocab; log-softmax reduces over the free (vocab)
dim locally.

Layouts are "transposed" throughout: activations are stored [channel, row]
with channels on partitions (4 k-tiles of 128) and rows = t*4+b on the free
dim, which feeds the PE directly both as stationary (lhsT) and moving
operands without any in-loop transposes.
"""

import os
import sys

import numpy as np

for _p in ("/opt/trn_rl_repo", os.path.expanduser("~/.axon_site/_ro/trn_rl_repo")):
    if os.path.isdir(_p) and _p not in sys.path:
        sys.path.insert(0, _p)

from contextlib import ExitStack

import concourse.bass as bass
import concourse.mybir as mybir
import concourse.tile as tile
from concourse import bacc
from concourse.bass import IndirectOffsetOnAxis, ds, ts
from concourse.bass_utils import run_bass_kernel_spmd
from concourse.masks import make_identity

F16 = mybir.dt.float16
F32 = mybir.dt.float32
U32 = mybir.dt.uint32
AF = mybir.ActivationFunctionType
ALU = mybir.AluOpType

NCORES = 8
B, T, S, H, D, V, L = 32, 64, 64, 512, 512, 32000, 2
BL = B // NCORES          # batch per core = 4
R = T * BL                # rows per core = 256
G4 = 4 * H                # 2048 gate rows
KT = H // 128             # 4 contraction k-tiles
MT = G4 // 128            # 16 gate m-tiles
LAG = 8                   # layer-2 lag (G1 batch granularity)
VSUP = 1024               # vocab superchunk (psum: [128,1024] f32 = 2 banks)
VSUB = 512                # matmul free-dim chunk (1 bank)


def _quad(m):
    """gate m-tile -> column quad in the [i|f|o|g] step-gate layout."""
    if m < 8:
        return m          # i (0-3), f (4-7)
    if m < 12:
        return m + 4      # g -> quads 12-15
    return m - 4          # o -> quads 8-11


def build_program():
    nc = bacc.Bacc(
        "TRN2",
        target_bir_lowering=False,
        debug=False,
        enable_asserts=False,
        num_devices=NCORES,
    )

    # ---- DRAM I/O ----
    d_embT = nc.dram_tensor("embT_in", [128, KT, R], F16, kind="ExternalInput")
    d_whhT = nc.dram_tensor("whhT", [L, H, G4], F16, kind="ExternalInput")
    d_wihT = nc.dram_tensor("wihT", [L, H, G4], F16, kind="ExternalInput")
    d_gbias = nc.dram_tensor("gbias", [L, 128, MT], F32, kind="ExternalInput")
    d_h0T = nc.dram_tensor("h0T", [L, 128, KT * BL], F16, kind="ExternalInput")
    d_c0T = nc.dram_tensor("c0T", [L, 128, KT * BL], F32, kind="ExternalInput")
    d_wainT = nc.dram_tensor("wattn_inT", [H, H], F16, kind="ExternalInput")
    d_waoutT = nc.dram_tensor("wattn_outT", [2 * H, H], F16, kind="ExternalInput")
    d_ctx = nc.dram_tensor("ctx", [BL, S, H], F16, kind="ExternalInput")
    d_ctxT = nc.dram_tensor("ctxT", [BL, H, S], F16, kind="ExternalInput")
    d_wlinT = nc.dram_tensor("wlinT", [H, V], F16, kind="ExternalInput")
    d_blin = nc.dram_tensor("blin", [1, V], F16, kind="ExternalInput")

    debug = os.environ.get("KERNEL_DEBUG", "0") == "1"
    if debug:
        d_dbg_g0 = nc.dram_tensor("dbg_g0", [128, MT, R], F32, kind="ExternalOutput")
        d_dbg_h1 = nc.dram_tensor(
            "dbg_h1", [128, KT, BL * (T + 1)], F32, kind="ExternalOutput"
        )
    d_logp = nc.dram_tensor("logp", [R, V], F32, kind="ExternalOutput")
    d_hout = nc.dram_tensor("hT_out", [L, 128, KT * BL], F32, kind="ExternalOutput")
    d_cout = nc.dram_tensor("cT_out", [L, 128, KT * BL], F32, kind="ExternalOutput")
    d_attn = nc.dram_tensor("attn_out", [BL, S], F32, kind="ExternalOutput")

    with tile.TileContext(nc) as tc, ExitStack() as octx:
        # ---------- persistent pool ----------
        pers = octx.enter_context(tc.tile_pool(name="pers", bufs=1))
        outsT = pers.tile([128, KT, R], F16)            # attention outputs^T
        hist1 = pers.tile([128, KT, BL * (T + 1)], F16)  # layer-1 h history^T
        hist2 = pers.tile([128, KT, BL * (T + 1)], F16)  # layer-2 h history^T
        ident = pers.tile([128, 128], F16)
        ones_row = pers.tile([1, 128], F16)
        make_identity(nc, ident[:])
        nc.gpsimd.memset(ones_row[:], 1.0)

        # ================= recurrence scope =================
        with ExitStack() as ctx:
            wpool = ctx.enter_context(tc.tile_pool(name="wts", bufs=1))
            gpool = ctx.enter_context(tc.tile_pool(name="gates", bufs=1))
            embp = ctx.enter_context(tc.tile_pool(name="emb", bufs=1))
            ewp = ctx.enter_context(tc.tile_pool(name="ew", bufs=4))
            cpool = ctx.enter_context(tc.tile_pool(name="cst", bufs=2))

            # ---- load weights / biases ----
            whh = []
            wih = []
            for l in range(L):
                wt = wpool.tile([128, KT, G4], F16, tag=f"whh{l}")
                nc.sync.dma_start(
                    wt[:], d_whhT[l].rearrange("(k p) m -> p k m", p=128)
                )
                whh.append(wt)
                wt2 = wpool.tile([128, KT, G4], F16, tag=f"wih{l}")
                nc.sync.dma_start(
                    wt2[:], d_wihT[l].rearrange("(k p) m -> p k m", p=128)
                )
                wih.append(wt2)
            gbias = wpool.tile([128, L, MT], F32, tag="gbias")
            nc.sync.dma_start(gbias[:], d_gbias.rearrange("l p m -> p l m"))

            # ---- embedding (host-gathered, transposed) ----
            g0t = gpool.tile([128, MT, R], F32, tag="g0")
            g1t = gpool.tile([128, MT, R], F32, tag="g1")
            embT = embp.tile([128, KT, R], F16, tag="embT")
            nc.sync.dma_start(embT[:], d_embT[:])
            with ExitStack() as ectx:
                ps_g0 = ectx.enter_context(
                    tc.tile_pool(name="ps_g0", bufs=4, space="PSUM")
                )
                # ---- G0 = Wih0 @ embT + bias (all steps batched) ----
                for m in range(MT):
                    ps = ps_g0.tile([128, R], F32, tag="psg")
                    for k in range(KT):
                        nc.tensor.matmul(
                            ps[:],
                            wih[0][:, k, ts(m, 128)],
                            embT[:, k, :],
                            start=(k == 0),
                            stop=(k == KT - 1),
                        )
                    nc.scalar.activation(
                        g0t[:, _quad(m), :],
                        ps[:],
                        AF.Identity,
                        bias=gbias[:, 0, m : m + 1],
                    )

            # ---- init recurrent state ----
            cprev = []
            for l in range(L):
                nc.sync.dma_start(
                    (hist1 if l == 0 else hist2)[:, :, 0:BL],
                    d_h0T[l].rearrange("p (k b) -> p k b", b=BL),
                )
                ct = cpool.tile([128, KT * BL], F32, tag=f"c{l}")
                nc.sync.dma_start(ct[:], d_c0T[l])
                cprev.append(ct)

            rctx = ctx.enter_context(ExitStack())
            ps_l = [
                rctx.enter_context(
                    tc.tile_pool(name=f"ps_l{l}", bufs=3, space="PSUM")
                )
                for l in range(L)
            ]
            ps_g = rctx.enter_context(tc.tile_pool(name="ps_g1", bufs=2, space="PSUM"))

            def lstm_step(l, t):
                hist = hist1 if l == 0 else hist2
                gsrc = g0t if l == 0 else g1t
                ps = ps_l[l].tile([128, 16 * BL], F32, tag=f"psr{l}")
                first = True
                for m in range(MT):
                    for k in range(KT):
                        nc.tensor.matmul(
                            ps[:, ds(BL * _quad(m), BL)],
                            whh[l][:, k, ts(m, 128)],
                            hist[:, k, ds(BL * t, BL)],
                            start=first,
                            stop=(m == MT - 1 and k == KT - 1),
                        )
                        first = False
                sb = ewp.tile([128, 16 * BL], F32, tag=f"sb{l}")
                nc.vector.tensor_tensor(
                    sb[:].rearrange("p (q b) -> p q b", b=BL),
                    ps[:].rearrange("p (q b) -> p q b", b=BL),
                    gsrc[:, :, ds(BL * t, BL)],
                    op=ALU.add,
                )
                ga = ewp.tile([128, 16 * BL], F32, tag=f"ga{l}")
                nc.scalar.activation(ga[:, 0 : 12 * BL], sb[:, 0 : 12 * BL], AF.Sigmoid)
                nc.scalar.activation(
                    ga[:, 12 * BL : 16 * BL], sb[:, 12 * BL : 16 * BL], AF.Tanh
                )
                nch = KT * BL  # 16 = one [128, k, b] channel block
                ig = ewp.tile([128, nch], F32, tag=f"ig{l}")
                nc.vector.tensor_tensor(
                    ig[:], ga[:, 12 * BL : 16 * BL], ga[:, 0:nch], op=ALU.mult
                )
                fc = ewp.tile([128, nch], F32, tag=f"fc{l}")
                nc.vector.tensor_tensor(
                    fc[:], ga[:, nch : 2 * nch], cprev[l][:], op=ALU.mult
                )
                cn = cpool.tile([128, nch], F32, tag=f"c{l}")
                nc.vector.tensor_tensor(cn[:], ig[:], fc[:], op=ALU.add)
                tcn = ewp.tile([128, nch], F32, tag=f"tc{l}")
                nc.scalar.activation(tcn[:], cn[:], AF.Tanh)
                nc.vector.tensor_tensor(
                    hist[:, :, ds(BL * (t + 1), BL)],
                    ga[:, 2 * nch : 3 * nch].rearrange("p (k b) -> p k b", b=BL),
                    tcn[:].rearrange("p (k b) -> p k b", b=BL),
                    op=ALU.mult,
                )
                cprev[l] = cn

            # ---- interleaved recurrences: layer 2 lags by LAG steps ----
            for tt in range(T + LAG):
                if tt < T:
                    lstm_step(0, tt)
                if tt < T and tt % LAG == LAG - 1:
                    t0 = tt - LAG + 1
                    for m in range(MT):
                        ps = ps_g.tile([128, BL * LAG], F32, tag="psg1")
                        for k in range(KT):
                            nc.tensor.matmul(
                                ps[:],
                                wih[1][:, k, ts(m, 128)],
                                hist1[:, k, ds(BL * (t0 + 1), BL * LAG)],
                                start=(k == 0),
                                stop=(k == KT - 1),
                            )
                        nc.scalar.activation(
                            g1t[:, _quad(m), ds(BL * t0, BL * LAG)],
                            ps[:],
                            AF.Identity,
                            bias=gbias[:, 1, m : m + 1],
                        )
                if tt >= LAG:
                    lstm_step(1, tt - LAG)

            if debug:
                nc.gpsimd.dma_start(d_dbg_g0[:], g0t[:])
                nc.gpsimd.dma_start(d_dbg_h1[:], hist1[:])

            # ---- final h/c state outputs ----
            for l in range(L):
                hist = hist1 if l == 0 else hist2
                nc.gpsimd.dma_start(d_hout[l], hist[:, :, ds(BL * T, BL)])
                nc.sync.dma_start(d_cout[l], cprev[l][:])
            rctx.close()

            # ================= attention (batched over t) =================
            wainT = wpool.tile([128, KT, H], F16, tag="wainT")
            nc.sync.dma_start(wainT[:], d_wainT.rearrange("(k p) m -> p k m", p=128))
            waoutT = wpool.tile([128, 2 * KT, H], F16, tag="waoutT")
            nc.sync.dma_start(
                waoutT[:], d_waoutT.rearrange("(k p) m -> p k m", p=128)
            )
            ctx_sb = wpool.tile([S, BL, H], F16, tag="ctx")
            nc.sync.dma_start(ctx_sb[:], d_ctx.rearrange("b s h -> s b h"))
            ctxT_sb = wpool.tile([128, KT, BL, S], F16, tag="ctxT")
            for b in range(BL):
                nc.sync.dma_start(
                    ctxT_sb[:, :, b, :],
                    d_ctxT[b].rearrange("(k p) s -> p k s", p=128),
                )

            actx = ctx.enter_context(ExitStack())
            ps_q = actx.enter_context(tc.tile_pool(name="ps_q", bufs=2, space="PSUM"))
            ps_tr = actx.enter_context(tc.tile_pool(name="ps_at", bufs=2, space="PSUM"))

            # qT = W_attn_in @ h2T   [H, R]
            qT = embp.tile([128, KT, R], F16, tag="qT")
            for m in range(KT):
                ps = ps_q.tile([128, R], F32, tag="psq")
                for k in range(KT):
                    nc.tensor.matmul(
                        ps[:],
                        wainT[:, k, ts(m, 128)],
                        hist2[:, k, ds(BL, R)],
                        start=(k == 0),
                        stop=(k == KT - 1),
                    )
                nc.vector.tensor_copy(qT[:, m, :], ps[:])

            # per-batch: scores -> softmax -> attnT -> ctx^T
            ctxcat = embp.tile([128, KT, R], F16, tag="ctxcat")
            q4 = qT[:].rearrange("p k (t b) -> p k t b", b=BL)
            cc4 = ctxcat[:].rearrange("p k (t b) -> p k t b", b=BL)
            for b in range(BL):
                ps_sc = ps_tr.tile([T, S], F32, tag="ps_sc")
                for k in range(KT):
                    nc.tensor.matmul(
                        ps_sc[:],
                        q4[:, k, :, b],
                        ctxT_sb[:, k, b, :],
                        start=(k == 0),
                        stop=(k == KT - 1),
                    )
                exps = ewp.tile([T, S], F16, tag="exps")
                sume = ewp.tile([T, 1], F32, tag="sume")
                nc.scalar.activation(exps[:], ps_sc[:], AF.Exp, accum_out=sume[:])
                rsum = ewp.tile([T, 1], F32, tag="rsum")
                nc.vector.reciprocal(rsum[:], sume[:])
                attnw = ewp.tile([T, S], F16, tag="attnw")
                nc.vector.tensor_scalar_mul(attnw[:], exps[:], rsum[:])
                # attn_last output (t = T-1)
                nc.gpsimd.dma_start(d_attn[b : b + 1, :], attnw[T - 1 : T, :])
                # transpose attn -> [S, T]
                ps_at = ps_tr.tile([S, T], F16, tag="ps_at")
                nc.tensor.transpose(ps_at[:], attnw[:], ident[0:T, 0:T])
                attnT = ewp.tile([S, T], F16, tag="attnT")
                nc.vector.tensor_copy(attnT[:], ps_at[:])
                # ctx^T[b] = context_b^T @ attnT  -> scatter into ctxcat cols b::BL
                for m in range(KT):
                    ps_cx = ps_tr.tile([128, T], F32, tag="ps_cx")
                    nc.tensor.matmul(
                        ps_cx[:],
                        ctx_sb[:, b, ts(m, 128)],
                        attnT[:],
                        start=True,
                        stop=True,
                    )
                    nc.vector.tensor_copy(cc4[:, m, :, b], ps_cx[:])

            # outsT = tanh(W_attn_out @ [ctx; h2]^T)
            for m in range(KT):
                ps = ps_q.tile([128, R], F32, tag="psq")
                for k in range(2 * KT):
                    rhs = (
                        ctxcat[:, k, :]
                        if k < KT
                        else hist2[:, k - KT, ds(BL, R)]
                    )
                    nc.tensor.matmul(
                        ps[:],
                        waoutT[:, k, ts(m, 128)],
                        rhs,
                        start=(k == 0),
                        stop=(k == 2 * KT - 1),
                    )
                nc.scalar.activation(outsT[:, m, :], ps[:], AF.Tanh)

        # ================= projection + log-softmax =================
        with ExitStack() as ctx:
            wsp = ctx.enter_context(tc.tile_pool(name="wstream", bufs=3))
            bsp = ctx.enter_context(tc.tile_pool(name="bstream", bufs=3))
            expp = ctx.enter_context(tc.tile_pool(name="exp", bufs=1))
            accp = ctx.enter_context(tc.tile_pool(name="acc", bufs=1))
            outp = ctx.enter_context(tc.tile_pool(name="outp", bufs=3))
            ps_p = ctx.enter_context(tc.tile_pool(name="ps_p", bufs=3, space="PSUM"))

            exp_sb = expp.tile([128, 2, V], F16)
            sacc = accp.tile([128, 64], F32)
            wlin4 = d_wlinT.rearrange("(k p) v -> p k v", p=128)

            sups = []
            v0 = 0
            while v0 < V:
                sups.append((v0, min(VSUP, V - v0)))
                v0 += VSUP

            for si, (v0, vlen) in enumerate(sups):
                wt = wsp.tile([128, KT, VSUP], F16, tag="wt")
                nc.sync.dma_start(wt[:, :, 0:vlen], wlin4[:, :, ds(v0, vlen)])
                bt = bsp.tile([1, VSUP], F16, tag="bt")
                nc.sync.dma_start(bt[:, 0:vlen], d_blin[:, ds(v0, vlen)])
                for m in range(2):
                    ps = ps_p.tile([128, VSUP], F32, tag="psp")
                    for j0 in range(0, vlen, VSUB):
                        jl = min(VSUB, vlen - j0)
                        nc.tensor.matmul(
                            ps[:, ds(j0, jl)],
                            ones_row[:, 0:128],
                            bt[:, ds(j0, jl)],
                            start=True,
                            stop=False,
                        )
                        for k in range(KT):
                            nc.tensor.matmul(
                                ps[:, ds(j0, jl)],
                                outsT[:, k, ts(m, 128)],
                                wt[:, k, ds(j0, jl)],
                                start=False,
                                stop=(k == KT - 1),
                            )
                    nc.scalar.activation(
                        exp_sb[:, m, ds(v0, vlen)],
                        ps[:, 0:vlen],
                        AF.Exp,
                        accum_out=sacc[:, 32 * m + si : 32 * m + si + 1],
                    )

            rz = accp.tile([128, 2], F32)
            for m in range(2):
                tot = accp.tile([128, 1], F32, tag="tot")
                nc.vector.reduce_sum(
                    tot[:], sacc[:, ds(32 * m, len(sups))], axis=mybir.AxisListType.X
                )
                nc.vector.reciprocal(rz[:, m : m + 1], tot[:])

            for si, (v0, vlen) in enumerate(sups):
                for m in range(2):
                    ot = outp.tile([128, VSUP], F32, tag="ot")
                    nc.scalar.activation(
                        ot[:, 0:vlen],
                        exp_sb[:, m, ds(v0, vlen)],
                        AF.Ln,
                        scale=rz[:, m : m + 1],
                    )
                    nc.sync.dma_start(
                        d_logp[ds(128 * m, 128), ds(v0, vlen)], ot[:, 0:vlen]
                    )

    nc.compile()
    return nc


_NC_CACHE = {}


def _get_nc():
    if "nc" not in _NC_CACHE:
        _NC_CACHE["nc"] = build_program()
    return _NC_CACHE["nc"]


def kernel(
    tokens,
    h0,
    c0,
    context,
    embedding,
    Wih,
    Whh,
    bih,
    bhh,
    W_attn_in,
    W_attn_out,
    W_lin,
    b_lin,
):
    tokens = np.asarray(tokens)
    h0 = np.asarray(h0, dtype=np.float32)
    c0 = np.asarray(c0, dtype=np.float32)
    context = np.asarray(context, dtype=np.float32)
    embedding = np.asarray(embedding, dtype=np.float32)
    Wih = np.asarray(Wih, dtype=np.float32)
    Whh = np.asarray(Whh, dtype=np.float32)
    bih = np.asarray(bih, dtype=np.float32)
    bhh = np.asarray(bhh, dtype=np.float32)
    W_attn_in = np.asarray(W_attn_in, dtype=np.float32)
    W_attn_out = np.asarray(W_attn_out, dtype=np.float32)
    W_lin = np.asarray(W_lin, dtype=np.float32)
    b_lin = np.asarray(b_lin, dtype=np.float32)

    f16 = np.float16
    emb16 = embedding.astype(f16)
    WhhT = np.ascontiguousarray(np.transpose(Whh, (0, 2, 1)).astype(f16))
    WihT = np.ascontiguousarray(np.transpose(Wih, (0, 2, 1)).astype(f16))
    gbias = np.ascontiguousarray(
        (bih + bhh).astype(np.float32).reshape(L, MT, 128).transpose(0, 2, 1)
    )
    WaInT = np.ascontiguousarray(W_attn_in.T.astype(f16))
    WaOutT = np.ascontiguousarray(W_attn_out.T.astype(f16))
    WlinT = np.ascontiguousarray(W_lin.T.astype(f16))
    blin = np.ascontiguousarray(b_lin.astype(f16).reshape(1, V))

    def pack_T(x):  # [BL, 512] -> [128, KT*BL] transposed k-major packing
        return np.ascontiguousarray(
            x.T.reshape(KT, 128, BL).transpose(1, 0, 2).reshape(128, KT * BL)
        )

    in_maps = []
    for core in range(NCORES):
        bs = slice(BL * core, BL * (core + 1))
        tok = tokens[bs, :].astype(np.int64)
        tokflat = tok.T.reshape(-1)  # rows r = t*BL + b
        embT_in = np.ascontiguousarray(
            emb16[tokflat].reshape(R, KT, 128).transpose(2, 1, 0)
        )
        h0T = np.stack([pack_T(h0[l, bs, :]) for l in range(L)]).astype(f16)
        c0T = np.stack([pack_T(c0[l, bs, :]) for l in range(L)]).astype(np.float32)
        ctxl = np.ascontiguousarray(context[bs].astype(f16))
        ctxTl = np.ascontiguousarray(np.transpose(context[bs], (0, 2, 1)).astype(f16))
        in_maps.append(
            dict(
                embT_in=embT_in,
                whhT=WhhT,
                wihT=WihT,
                gbias=gbias,
                h0T=h0T,
                c0T=c0T,
                wattn_inT=WaInT,
                wattn_outT=WaOutT,
                ctx=ctxl,
                ctxT=ctxTl,
                wlinT=WlinT,
                blin=blin,
            )
        )

    nc = _get_nc()
    trace = os.environ.get("KERNEL_TRACE", "0") == "1"
    res = run_bass_kernel_spmd(
        nc, in_maps, core_ids=list(range(NCORES)), trace=trace
    )
    _NC_CACHE["last_result"] = res
    outs = res.results

    log_probs = np.empty((T, B, V), np.float32)
    hT = np.empty((L, B, H), np.float32)
    cT = np.empty((L, B, H), np.float32)
    attn_last = np.empty((B, S), np.float32)

    def unpack_T(x):  # [128, KT*BL] -> [BL, 512]
        return (
            x.reshape(128, KT, BL).transpose(2, 1, 0).reshape(BL, H)
        )

    for core in range(NCORES):
        o = outs[core]
        bs = slice(BL * core, BL * (core + 1))
        log_probs[:, bs, :] = o["logp"].reshape(T, BL, V)
        for l in range(L):
            hT[l, bs, :] = unpack_T(o["hT_out"][l])
            cT[l, bs, :] = unpack_T(o["cT_out"][l])
        attn_last[bs, :] = o["attn_out"]

    return log_probs, hT, cT, attn_last


# revision 20
# speedup vs baseline: 1.0448x; 1.0448x over previous
"""Trainium2 Bass kernel for nn_Decoder (2-layer LSTM decoder + Luong attention
+ vocab projection + log-softmax), 8-way batch-parallel SPMD.

Sharding: data-parallel over batch (B=32 -> 4 per core). The LSTM recurrence,
attention, projection and log-softmax for a core's 4 batch elements are fully
local, so no collectives are needed. Each core's projection covers all
T*B_local = 256 rows x full vocab; log-softmax reduces over the free (vocab)
dim locally.

Layouts are "transposed" throughout: activations are stored [channel, row]
with channels on partitions (4 k-tiles of 128) and rows = t*4+b on the free
dim, which feeds the PE directly both as stationary (lhsT) and moving
operands without any in-loop transposes.
"""

import os
import sys

import numpy as np

for _p in ("/opt/trn_rl_repo", os.path.expanduser("~/.axon_site/_ro/trn_rl_repo")):
    if os.path.isdir(_p) and _p not in sys.path:
        sys.path.insert(0, _p)

from contextlib import ExitStack

import concourse.bass as bass
import concourse.mybir as mybir
import concourse.tile as tile
from concourse import bacc
from concourse.bass import IndirectOffsetOnAxis, ds, ts
from concourse.bass_utils import run_bass_kernel_spmd
from concourse.masks import make_identity

F16 = mybir.dt.float16
F32 = mybir.dt.float32
F8 = mybir.dt.float8e4
U32 = mybir.dt.uint32
AF = mybir.ActivationFunctionType
ALU = mybir.AluOpType

NCORES = 8
B, T, S, H, D, V, L = 32, 64, 64, 512, 512, 32000, 2
BL = B // NCORES          # batch per core = 4
R = T * BL                # rows per core = 256
G4 = 4 * H                # 2048 gate rows
KT = H // 128             # 4 contraction k-tiles
MT = G4 // 128            # 16 gate m-tiles
LAG = 8                   # layer-2 lag (G1 batch granularity)
VSUP = 1024               # vocab superchunk (psum: [128,1024] f32 = 2 banks)
VSUB = 512                # matmul free-dim chunk (1 bank)


def _quad(m):
    """gate m-tile -> column quad in the [i|f|o|g] step-gate layout."""
    if m < 8:
        return m          # i (0-3), f (4-7)
    if m < 12:
        return m + 4      # g -> quads 12-15
    return m - 4          # o -> quads 8-11


def build_program():
    nc = bacc.Bacc(
        "TRN2",
        target_bir_lowering=False,
        debug=False,
        enable_asserts=False,
        num_devices=NCORES,
    )

    # ---- DRAM I/O ----
    d_embT = nc.dram_tensor("embT_in", [128, KT, R], F16, kind="ExternalInput")
    d_whhT = nc.dram_tensor("whhT", [L, H, G4], F16, kind="ExternalInput")
    d_wihT = nc.dram_tensor("wihT", [L, H, G4], F16, kind="ExternalInput")
    d_gbias = nc.dram_tensor("gbias", [L, 128, MT], F32, kind="ExternalInput")
    d_h0T = nc.dram_tensor("h0T", [L, 128, KT * BL], F16, kind="ExternalInput")
    d_c0T = nc.dram_tensor("c0T", [L, 128, KT * BL], F32, kind="ExternalInput")
    d_wainT = nc.dram_tensor("wattn_inT", [H, H], F16, kind="ExternalInput")
    d_waoutT = nc.dram_tensor("wattn_outT", [2 * H, H], F16, kind="ExternalInput")
    d_ctx = nc.dram_tensor("ctx", [BL, S, H], F16, kind="ExternalInput")
    d_ctxT = nc.dram_tensor("ctxT", [BL, H, S], F16, kind="ExternalInput")
    d_wlinT = nc.dram_tensor("wlinT", [H, V], F8, kind="ExternalInput")
    d_blin = nc.dram_tensor("blin", [1, V], F16, kind="ExternalInput")

    debug = os.environ.get("KERNEL_DEBUG", "0") == "1"
    if debug:
        d_dbg_g0 = nc.dram_tensor("dbg_g0", [128, MT, R], F32, kind="ExternalOutput")
        d_dbg_h1 = nc.dram_tensor(
            "dbg_h1", [128, KT, BL * (T + 1)], F32, kind="ExternalOutput"
        )
    d_logp = nc.dram_tensor("logp", [R, V], F16, kind="ExternalOutput")
    d_hout = nc.dram_tensor("hT_out", [L, 128, KT * BL], F32, kind="ExternalOutput")
    d_cout = nc.dram_tensor("cT_out", [L, 128, KT * BL], F32, kind="ExternalOutput")
    d_attn = nc.dram_tensor("attn_out", [BL, S], F32, kind="ExternalOutput")

    with tile.TileContext(nc) as tc, ExitStack() as octx:
        # ---------- persistent pool ----------
        pers = octx.enter_context(tc.tile_pool(name="pers", bufs=1))
        outsT = pers.tile([128, KT, R], F16)            # attention outputs^T
        hist1 = pers.tile([128, KT, BL * (T + 1)], F16)  # layer-1 h history^T
        hist2 = pers.tile([128, KT, BL * (T + 1)], F16)  # layer-2 h history^T
        ident = pers.tile([128, 128], F16)
        ones_row = pers.tile([1, 128], F16)
        make_identity(nc, ident[:])
        nc.gpsimd.memset(ones_row[:], 1.0)

        # ================= recurrence scope =================
        with ExitStack() as ctx:
            wpool = ctx.enter_context(tc.tile_pool(name="wts", bufs=1))
            gpool = ctx.enter_context(tc.tile_pool(name="gates", bufs=1))
            embp = ctx.enter_context(tc.tile_pool(name="emb", bufs=1))
            ewp = ctx.enter_context(tc.tile_pool(name="ew", bufs=4))
            cpool = ctx.enter_context(tc.tile_pool(name="cst", bufs=2))

            # ---- load weights / biases ----
            whh = []
            wih = []
            for l in range(L):
                wt = wpool.tile([128, KT, G4], F16, tag=f"whh{l}")
                nc.sync.dma_start(
                    wt[:], d_whhT[l].rearrange("(k p) m -> p k m", p=128)
                )
                whh.append(wt)
                wt2 = wpool.tile([128, KT, G4], F16, tag=f"wih{l}")
                nc.sync.dma_start(
                    wt2[:], d_wihT[l].rearrange("(k p) m -> p k m", p=128)
                )
                wih.append(wt2)
            gbias = wpool.tile([128, L, MT], F32, tag="gbias")
            nc.sync.dma_start(gbias[:], d_gbias.rearrange("l p m -> p l m"))

            # ---- embedding (host-gathered, transposed) ----
            g0t = gpool.tile([128, MT, R], F32, tag="g0")
            g1t = gpool.tile([128, MT, R], F32, tag="g1")
            embT = embp.tile([128, KT, R], F16, tag="embT")
            nc.sync.dma_start(embT[:], d_embT[:])
            with ExitStack() as ectx:
                ps_g0 = ectx.enter_context(
                    tc.tile_pool(name="ps_g0", bufs=4, space="PSUM")
                )
                # ---- G0 = Wih0 @ embT + bias (all steps batched) ----
                for m in range(MT):
                    ps = ps_g0.tile([128, R], F32, tag="psg")
                    for k in range(KT):
                        nc.tensor.matmul(
                            ps[:],
                            wih[0][:, k, ts(m, 128)],
                            embT[:, k, :],
                            start=(k == 0),
                            stop=(k == KT - 1),
                        )
                    nc.scalar.activation(
                        g0t[:, _quad(m), :],
                        ps[:],
                        AF.Identity,
                        bias=gbias[:, 0, m : m + 1],
                    )

            # ---- init recurrent state ----
            cprev = []
            for l in range(L):
                nc.sync.dma_start(
                    (hist1 if l == 0 else hist2)[:, :, 0:BL],
                    d_h0T[l].rearrange("p (k b) -> p k b", b=BL),
                )
                ct = cpool.tile([128, KT * BL], F32, tag=f"c{l}")
                nc.sync.dma_start(ct[:], d_c0T[l])
                cprev.append(ct)

            rctx = ctx.enter_context(ExitStack())
            ps_l = [
                rctx.enter_context(
                    tc.tile_pool(name=f"ps_l{l}", bufs=3, space="PSUM")
                )
                for l in range(L)
            ]
            ps_g = rctx.enter_context(tc.tile_pool(name="ps_g1", bufs=2, space="PSUM"))

            def lstm_step(l, t):
                hist = hist1 if l == 0 else hist2
                gsrc = g0t if l == 0 else g1t
                ps = ps_l[l].tile([128, 16 * BL], F32, tag=f"psr{l}")
                first = True
                for m in range(MT):
                    for k in range(KT):
                        nc.tensor.matmul(
                            ps[:, ds(BL * _quad(m), BL)],
                            whh[l][:, k, ts(m, 128)],
                            hist[:, k, ds(BL * t, BL)],
                            start=first,
                            stop=(m == MT - 1 and k == KT - 1),
                        )
                        first = False
                sb = ewp.tile([128, 16 * BL], F32, tag=f"sb{l}")
                nc.vector.tensor_tensor(
                    sb[:].rearrange("p (q b) -> p q b", b=BL),
                    ps[:].rearrange("p (q b) -> p q b", b=BL),
                    gsrc[:, :, ds(BL * t, BL)],
                    op=ALU.add,
                )
                ga = ewp.tile([128, 16 * BL], F32, tag=f"ga{l}")
                nc.scalar.activation(ga[:, 0 : 12 * BL], sb[:, 0 : 12 * BL], AF.Sigmoid)
                nc.scalar.activation(
                    ga[:, 12 * BL : 16 * BL], sb[:, 12 * BL : 16 * BL], AF.Tanh
                )
                nch = KT * BL  # 16 = one [128, k, b] channel block
                ig = ewp.tile([128, nch], F32, tag=f"ig{l}")
                nc.vector.tensor_tensor(
                    ig[:], ga[:, 12 * BL : 16 * BL], ga[:, 0:nch], op=ALU.mult
                )
                fc = ewp.tile([128, nch], F32, tag=f"fc{l}")
                nc.vector.tensor_tensor(
                    fc[:], ga[:, nch : 2 * nch], cprev[l][:], op=ALU.mult
                )
                cn = cpool.tile([128, nch], F32, tag=f"c{l}")
                nc.vector.tensor_tensor(cn[:], ig[:], fc[:], op=ALU.add)
                tcn = ewp.tile([128, nch], F32, tag=f"tc{l}")
                nc.scalar.activation(tcn[:], cn[:], AF.Tanh)
                nc.vector.tensor_tensor(
                    hist[:, :, ds(BL * (t + 1), BL)],
                    ga[:, 2 * nch : 3 * nch].rearrange("p (k b) -> p k b", b=BL),
                    tcn[:].rearrange("p (k b) -> p k b", b=BL),
                    op=ALU.mult,
                )
                cprev[l] = cn

            # ---- interleaved recurrences: layer 2 lags by LAG steps ----
            for tt in range(T + LAG):
                if tt < T:
                    lstm_step(0, tt)
                if tt < T and tt % LAG == LAG - 1:
                    t0 = tt - LAG + 1
                    for m in range(MT):
                        ps = ps_g.tile([128, BL * LAG], F32, tag="psg1")
                        for k in range(KT):
                            nc.tensor.matmul(
                                ps[:],
                                wih[1][:, k, ts(m, 128)],
                                hist1[:, k, ds(BL * (t0 + 1), BL * LAG)],
                                start=(k == 0),
                                stop=(k == KT - 1),
                            )
                        nc.scalar.activation(
                            g1t[:, _quad(m), ds(BL * t0, BL * LAG)],
                            ps[:],
                            AF.Identity,
                            bias=gbias[:, 1, m : m + 1],
                        )
                if tt >= LAG:
                    lstm_step(1, tt - LAG)

            if debug:
                nc.gpsimd.dma_start(d_dbg_g0[:], g0t[:])
                nc.gpsimd.dma_start(d_dbg_h1[:], hist1[:])

            # ---- final h/c state outputs ----
            for l in range(L):
                hist = hist1 if l == 0 else hist2
                nc.gpsimd.dma_start(d_hout[l], hist[:, :, ds(BL * T, BL)])
                nc.sync.dma_start(d_cout[l], cprev[l][:])
            rctx.close()

            # ================= attention (batched over t) =================
            wainT = wpool.tile([128, KT, H], F16, tag="wainT")
            nc.sync.dma_start(wainT[:], d_wainT.rearrange("(k p) m -> p k m", p=128))
            waoutT = wpool.tile([128, 2 * KT, H], F16, tag="waoutT")
            nc.sync.dma_start(
                waoutT[:], d_waoutT.rearrange("(k p) m -> p k m", p=128)
            )
            ctx_sb = wpool.tile([S, BL, H], F16, tag="ctx")
            nc.sync.dma_start(ctx_sb[:], d_ctx.rearrange("b s h -> s b h"))
            ctxT_sb = wpool.tile([128, KT, BL, S], F16, tag="ctxT")
            for b in range(BL):
                nc.sync.dma_start(
                    ctxT_sb[:, :, b, :],
                    d_ctxT[b].rearrange("(k p) s -> p k s", p=128),
                )

            actx = ctx.enter_context(ExitStack())
            ps_q = actx.enter_context(tc.tile_pool(name="ps_q", bufs=2, space="PSUM"))
            ps_tr = actx.enter_context(tc.tile_pool(name="ps_at", bufs=2, space="PSUM"))

            # qT = W_attn_in @ h2T   [H, R]
            qT = embp.tile([128, KT, R], F16, tag="qT")
            for m in range(KT):
                ps = ps_q.tile([128, R], F32, tag="psq")
                for k in range(KT):
                    nc.tensor.matmul(
                        ps[:],
                        wainT[:, k, ts(m, 128)],
                        hist2[:, k, ds(BL, R)],
                        start=(k == 0),
                        stop=(k == KT - 1),
                    )
                nc.vector.tensor_copy(qT[:, m, :], ps[:])

            # per-batch: scores -> softmax -> attnT -> ctx^T
            ctxcat = embp.tile([128, KT, R], F16, tag="ctxcat")
            q4 = qT[:].rearrange("p k (t b) -> p k t b", b=BL)
            cc4 = ctxcat[:].rearrange("p k (t b) -> p k t b", b=BL)
            for b in range(BL):
                ps_sc = ps_tr.tile([T, S], F32, tag="ps_sc")
                for k in range(KT):
                    nc.tensor.matmul(
                        ps_sc[:],
                        q4[:, k, :, b],
                        ctxT_sb[:, k, b, :],
                        start=(k == 0),
                        stop=(k == KT - 1),
                    )
                exps = ewp.tile([T, S], F16, tag="exps")
                sume = ewp.tile([T, 1], F32, tag="sume")
                nc.scalar.activation(exps[:], ps_sc[:], AF.Exp, accum_out=sume[:])
                rsum = ewp.tile([T, 1], F32, tag="rsum")
                nc.vector.reciprocal(rsum[:], sume[:])
                attnw = ewp.tile([T, S], F16, tag="attnw")
                nc.vector.tensor_scalar_mul(attnw[:], exps[:], rsum[:])
                # attn_last output (t = T-1)
                nc.gpsimd.dma_start(d_attn[b : b + 1, :], attnw[T - 1 : T, :])
                # transpose attn -> [S, T]
                ps_at = ps_tr.tile([S, T], F16, tag="ps_at")
                nc.tensor.transpose(ps_at[:], attnw[:], ident[0:T, 0:T])
                attnT = ewp.tile([S, T], F16, tag="attnT")
                nc.vector.tensor_copy(attnT[:], ps_at[:])
                # ctx^T[b] = context_b^T @ attnT  -> scatter into ctxcat cols b::BL
                for m in range(KT):
                    ps_cx = ps_tr.tile([128, T], F32, tag="ps_cx")
                    nc.tensor.matmul(
                        ps_cx[:],
                        ctx_sb[:, b, ts(m, 128)],
                        attnT[:],
                        start=True,
                        stop=True,
                    )
                    nc.vector.tensor_copy(cc4[:, m, :, b], ps_cx[:])

            # outsT = tanh(W_attn_out @ [ctx; h2]^T)
            for m in range(KT):
                ps = ps_q.tile([128, R], F32, tag="psq")
                for k in range(2 * KT):
                    rhs = (
                        ctxcat[:, k, :]
                        if k < KT
                        else hist2[:, k - KT, ds(BL, R)]
                    )
                    nc.tensor.matmul(
                        ps[:],
                        waoutT[:, k, ts(m, 128)],
                        rhs,
                        start=(k == 0),
                        stop=(k == 2 * KT - 1),
                    )
                nc.scalar.activation(outsT[:, m, :], ps[:], AF.Tanh)

        # ================= projection + log-softmax =================
        with ExitStack() as ctx:
            wsp = ctx.enter_context(tc.tile_pool(name="wstream", bufs=3))
            bsp = ctx.enter_context(tc.tile_pool(name="bstream", bufs=3))
            lgp = ctx.enter_context(tc.tile_pool(name="lgts", bufs=1))
            accp = ctx.enter_context(tc.tile_pool(name="acc", bufs=1))
            scrp = ctx.enter_context(tc.tile_pool(name="scr", bufs=2))
            outp = ctx.enter_context(tc.tile_pool(name="outp", bufs=3))
            ps_p = ctx.enter_context(tc.tile_pool(name="ps_p", bufs=3, space="PSUM"))

            logits_sb = lgp.tile([128, 2, V], F16)
            sacc = accp.tile([128, 64], F32)
            wlin4 = d_wlinT.rearrange("(k p) v -> p k v", p=128)

            sups = []
            v0 = 0
            while v0 < V:
                sups.append((v0, min(VSUP, V - v0)))
                v0 += VSUP

            for si, (v0, vlen) in enumerate(sups):
                wt = wsp.tile([128, KT, VSUP], F8, tag="wt")
                nc.sync.dma_start(wt[:, :, 0:vlen], wlin4[:, :, ds(v0, vlen)])
                bt = bsp.tile([1, VSUP], F16, tag="bt")
                nc.sync.dma_start(bt[:, 0:vlen], d_blin[:, ds(v0, vlen)])
                for m in range(2):
                    ps = ps_p.tile([128, VSUP], F32, tag="psp")
                    for j0 in range(0, vlen, VSUB):
                        jl = min(VSUB, vlen - j0)
                        nc.tensor.matmul(
                            ps[:, ds(j0, jl)],
                            ones_row[:, 0:128],
                            bt[:, ds(j0, jl)],
                            start=True,
                            stop=False,
                        )
                        for k in range(KT):
                            nc.tensor.matmul(
                                ps[:, ds(j0, jl)],
                                outsT[:, k, ts(m, 128)],
                                wt[:, k, ds(j0, jl)],
                                start=False,
                                stop=(k == KT - 1),
                            )
                    # sum of exp via ACT accumulate (exp values discarded)
                    scr = scrp.tile([128, VSUP], F16, tag="scr")
                    nc.scalar.activation(
                        scr[:, 0:vlen],
                        ps[:, 0:vlen],
                        AF.Exp,
                        accum_out=sacc[:, 32 * m + si : 32 * m + si + 1],
                    )
                    # keep the logits (fp16), alternating evac engine
                    if si % 2 == 0:
                        nc.vector.tensor_copy(
                            logits_sb[:, m, ds(v0, vlen)], ps[:, 0:vlen]
                        )
                    else:
                        nc.scalar.activation(
                            logits_sb[:, m, ds(v0, vlen)], ps[:, 0:vlen], AF.Identity
                        )

            nlz = accp.tile([128, 2], F32)
            for m in range(2):
                tot = accp.tile([128, 1], F32, tag="tot")
                nc.vector.reduce_sum(
                    tot[:], sacc[:, ds(32 * m, len(sups))], axis=mybir.AxisListType.X
                )
                lnz = accp.tile([128, 1], F32, tag="lnz")
                nc.scalar.activation(lnz[:], tot[:], AF.Ln)
                nc.vector.tensor_scalar_mul(nlz[:, m : m + 1], lnz[:], -1.0)

            for si, (v0, vlen) in enumerate(sups):
                for m in range(2):
                    ot = outp.tile([128, VSUP], F16, tag="ot")
                    nc.vector.tensor_scalar_add(
                        ot[:, 0:vlen],
                        logits_sb[:, m, ds(v0, vlen)],
                        nlz[:, m : m + 1],
                    )
                    nc.sync.dma_start(
                        d_logp[ds(128 * m, 128), ds(v0, vlen)], ot[:, 0:vlen]
                    )

    nc.compile()
    return nc


_NC_CACHE = {}


def _get_nc():
    if "nc" not in _NC_CACHE:
        _NC_CACHE["nc"] = build_program()
    return _NC_CACHE["nc"]


def kernel(
    tokens,
    h0,
    c0,
    context,
    embedding,
    Wih,
    Whh,
    bih,
    bhh,
    W_attn_in,
    W_attn_out,
    W_lin,
    b_lin,
):
    tokens = np.asarray(tokens)
    h0 = np.asarray(h0, dtype=np.float32)
    c0 = np.asarray(c0, dtype=np.float32)
    context = np.asarray(context, dtype=np.float32)
    embedding = np.asarray(embedding, dtype=np.float32)
    Wih = np.asarray(Wih, dtype=np.float32)
    Whh = np.asarray(Whh, dtype=np.float32)
    bih = np.asarray(bih, dtype=np.float32)
    bhh = np.asarray(bhh, dtype=np.float32)
    W_attn_in = np.asarray(W_attn_in, dtype=np.float32)
    W_attn_out = np.asarray(W_attn_out, dtype=np.float32)
    W_lin = np.asarray(W_lin, dtype=np.float32)
    b_lin = np.asarray(b_lin, dtype=np.float32)

    f16 = np.float16
    emb16 = embedding.astype(f16)
    WhhT = np.ascontiguousarray(np.transpose(Whh, (0, 2, 1)).astype(f16))
    WihT = np.ascontiguousarray(np.transpose(Wih, (0, 2, 1)).astype(f16))
    gbias = np.ascontiguousarray(
        (bih + bhh).astype(np.float32).reshape(L, MT, 128).transpose(0, 2, 1)
    )
    WaInT = np.ascontiguousarray(W_attn_in.T.astype(f16))
    WaOutT = np.ascontiguousarray(W_attn_out.T.astype(f16))
    import ml_dtypes

    WlinT = np.ascontiguousarray(W_lin.T.astype(ml_dtypes.float8_e4m3))
    blin = np.ascontiguousarray(b_lin.astype(f16).reshape(1, V))

    def pack_T(x):  # [BL, 512] -> [128, KT*BL] transposed k-major packing
        return np.ascontiguousarray(
            x.T.reshape(KT, 128, BL).transpose(1, 0, 2).reshape(128, KT * BL)
        )

    in_maps = []
    for core in range(NCORES):
        bs = slice(BL * core, BL * (core + 1))
        tok = tokens[bs, :].astype(np.int64)
        tokflat = tok.T.reshape(-1)  # rows r = t*BL + b
        embT_in = np.ascontiguousarray(
            emb16[tokflat].reshape(R, KT, 128).transpose(2, 1, 0)
        )
        h0T = np.stack([pack_T(h0[l, bs, :]) for l in range(L)]).astype(f16)
        c0T = np.stack([pack_T(c0[l, bs, :]) for l in range(L)]).astype(np.float32)
        ctxl = np.ascontiguousarray(context[bs].astype(f16))
        ctxTl = np.ascontiguousarray(np.transpose(context[bs], (0, 2, 1)).astype(f16))
        in_maps.append(
            dict(
                embT_in=embT_in,
                whhT=WhhT,
                wihT=WihT,
                gbias=gbias,
                h0T=h0T,
                c0T=c0T,
                wattn_inT=WaInT,
                wattn_outT=WaOutT,
                ctx=ctxl,
                ctxT=ctxTl,
                wlinT=WlinT,
                blin=blin,
            )
        )

    nc = _get_nc()
    trace = os.environ.get("KERNEL_TRACE", "0") == "1"
    res = run_bass_kernel_spmd(
        nc, in_maps, core_ids=list(range(NCORES)), trace=trace
    )
    _NC_CACHE["last_result"] = res
    outs = res.results

    log_probs = np.empty((T, B, V), np.float32)
    hT = np.empty((L, B, H), np.float32)
    cT = np.empty((L, B, H), np.float32)
    attn_last = np.empty((B, S), np.float32)

    def unpack_T(x):  # [128, KT*BL] -> [BL, 512]
        return (
            x.reshape(128, KT, BL).transpose(2, 1, 0).reshape(BL, H)
        )

    for core in range(NCORES):
        o = outs[core]
        bs = slice(BL * core, BL * (core + 1))
        log_probs[:, bs, :] = o["logp"].astype(np.float32).reshape(T, BL, V)
        for l in range(L):
            hT[l, bs, :] = unpack_T(o["hT_out"][l])
            cT[l, bs, :] = unpack_T(o["cT_out"][l])
        attn_last[bs, :] = o["attn_out"]

    return log_probs, hT, cT, attn_last


# revision 22
# speedup vs baseline: 1.0876x; 1.0409x over previous
"""Trainium2 Bass kernel for nn_Decoder (2-layer LSTM decoder + Luong attention
+ vocab projection + log-softmax), 8-way batch-parallel SPMD.

Sharding: data-parallel over batch (B=32 -> 4 per core). The LSTM recurrence,
attention, projection and log-softmax for a core's 4 batch elements are fully
local, so no collectives are needed. Each core's projection covers all
T*B_local = 256 rows x full vocab; log-softmax reduces over the free (vocab)
dim locally.

Layouts are "transposed" throughout: activations are stored [channel, row]
with channels on partitions (4 k-tiles of 128) and rows = t*4+b on the free
dim, which feeds the PE directly both as stationary (lhsT) and moving
operands without any in-loop transposes.
"""

import os
import sys

import numpy as np

for _p in ("/opt/trn_rl_repo", os.path.expanduser("~/.axon_site/_ro/trn_rl_repo")):
    if os.path.isdir(_p) and _p not in sys.path:
        sys.path.insert(0, _p)

from contextlib import ExitStack

import concourse.bass as bass
import concourse.mybir as mybir
import concourse.tile as tile
from concourse import bacc
from concourse.bass import IndirectOffsetOnAxis, ds, ts
from concourse.bass_utils import run_bass_kernel_spmd
from concourse.masks import make_identity

F16 = mybir.dt.float16
F32 = mybir.dt.float32
F8 = mybir.dt.float8e4
U32 = mybir.dt.uint32
AF = mybir.ActivationFunctionType
ALU = mybir.AluOpType

NCORES = 8
B, T, S, H, D, V, L = 32, 64, 64, 512, 512, 32000, 2
BL = B // NCORES          # batch per core = 4
R = T * BL                # rows per core = 256
G4 = 4 * H                # 2048 gate rows
KT = H // 128             # 4 contraction k-tiles
MT = G4 // 128            # 16 gate m-tiles
LAG = 8                   # layer-2 lag (G1 batch granularity)
VSUP = 2048               # vocab superchunk (psum: [128,2048] f32 = 4 banks)
VSUB = 512                # matmul free-dim chunk (1 bank)


def _quad(m):
    """gate m-tile -> column quad in the [i|f|o|g] step-gate layout."""
    if m < 8:
        return m          # i (0-3), f (4-7)
    if m < 12:
        return m + 4      # g -> quads 12-15
    return m - 4          # o -> quads 8-11


def build_program():
    nc = bacc.Bacc(
        "TRN2",
        target_bir_lowering=False,
        debug=False,
        enable_asserts=False,
        num_devices=NCORES,
    )

    # ---- DRAM I/O ----
    d_embT = nc.dram_tensor("embT_in", [128, KT, R], F16, kind="ExternalInput")
    d_whhT = nc.dram_tensor("whhT", [L, H, G4], F16, kind="ExternalInput")
    d_wihT = nc.dram_tensor("wihT", [L, H, G4], F16, kind="ExternalInput")
    d_gbias = nc.dram_tensor("gbias", [L, 128, MT], F32, kind="ExternalInput")
    d_h0T = nc.dram_tensor("h0T", [L, 128, KT * BL], F16, kind="ExternalInput")
    d_c0T = nc.dram_tensor("c0T", [L, 128, KT * BL], F32, kind="ExternalInput")
    d_wainT = nc.dram_tensor("wattn_inT", [H, H], F16, kind="ExternalInput")
    d_waoutT = nc.dram_tensor("wattn_outT", [2 * H, H], F16, kind="ExternalInput")
    d_ctx = nc.dram_tensor("ctx", [BL, S, H], F16, kind="ExternalInput")
    d_ctxT = nc.dram_tensor("ctxT", [BL, H, S], F16, kind="ExternalInput")
    d_wlinT = nc.dram_tensor("wlinT", [H, V], F8, kind="ExternalInput")
    d_blin = nc.dram_tensor("blin", [128, V], F8, kind="ExternalInput")

    debug = os.environ.get("KERNEL_DEBUG", "0") == "1"
    if debug:
        d_dbg_g0 = nc.dram_tensor("dbg_g0", [128, MT, R], F32, kind="ExternalOutput")
        d_dbg_h1 = nc.dram_tensor(
            "dbg_h1", [128, KT, BL * (T + 1)], F32, kind="ExternalOutput"
        )
    d_logp = nc.dram_tensor("logp", [R, V], F16, kind="ExternalOutput")
    d_hout = nc.dram_tensor("hT_out", [L, 128, KT * BL], F32, kind="ExternalOutput")
    d_cout = nc.dram_tensor("cT_out", [L, 128, KT * BL], F32, kind="ExternalOutput")
    d_attn = nc.dram_tensor("attn_out", [BL, S], F32, kind="ExternalOutput")

    with tile.TileContext(nc) as tc, ExitStack() as octx:
        # ---------- persistent pool ----------
        pers = octx.enter_context(tc.tile_pool(name="pers", bufs=1))
        outsT = pers.tile([128, KT, R], F16)            # attention outputs^T
        hist1 = pers.tile([128, KT, BL * (T + 1)], F16)  # layer-1 h history^T
        hist2 = pers.tile([128, KT, BL * (T + 1)], F16)  # layer-2 h history^T
        ident = pers.tile([128, 128], F16)
        ones_row = pers.tile([1, 128], F16)
        make_identity(nc, ident[:])
        nc.gpsimd.memset(ones_row[:], 1.0)

        # ================= recurrence scope =================
        with ExitStack() as ctx:
            wpool = ctx.enter_context(tc.tile_pool(name="wts", bufs=1))
            gpool = ctx.enter_context(tc.tile_pool(name="gates", bufs=1))
            embp = ctx.enter_context(tc.tile_pool(name="emb", bufs=1))
            ewp = ctx.enter_context(tc.tile_pool(name="ew", bufs=4))
            cpool = ctx.enter_context(tc.tile_pool(name="cst", bufs=2))

            # ---- load weights / biases ----
            whh = []
            wih = []
            for l in range(L):
                wt = wpool.tile([128, KT, G4], F16, tag=f"whh{l}")
                nc.sync.dma_start(
                    wt[:], d_whhT[l].rearrange("(k p) m -> p k m", p=128)
                )
                whh.append(wt)
                wt2 = wpool.tile([128, KT, G4], F16, tag=f"wih{l}")
                nc.sync.dma_start(
                    wt2[:], d_wihT[l].rearrange("(k p) m -> p k m", p=128)
                )
                wih.append(wt2)
            gbias = wpool.tile([128, L, MT], F32, tag="gbias")
            nc.sync.dma_start(gbias[:], d_gbias.rearrange("l p m -> p l m"))

            # ---- embedding (host-gathered, transposed) ----
            g0t = gpool.tile([128, MT, R], F32, tag="g0")
            g1t = gpool.tile([128, MT, R], F32, tag="g1")
            embT = embp.tile([128, KT, R], F16, tag="embT")
            nc.sync.dma_start(embT[:], d_embT[:])
            with ExitStack() as ectx:
                ps_g0 = ectx.enter_context(
                    tc.tile_pool(name="ps_g0", bufs=4, space="PSUM")
                )
                # ---- G0 = Wih0 @ embT + bias (all steps batched) ----
                for m in range(MT):
                    ps = ps_g0.tile([128, R], F32, tag="psg")
                    for k in range(KT):
                        nc.tensor.matmul(
                            ps[:],
                            wih[0][:, k, ts(m, 128)],
                            embT[:, k, :],
                            start=(k == 0),
                            stop=(k == KT - 1),
                        )
                    nc.scalar.activation(
                        g0t[:, _quad(m), :],
                        ps[:],
                        AF.Identity,
                        bias=gbias[:, 0, m : m + 1],
                    )

            # ---- init recurrent state ----
            cprev = []
            for l in range(L):
                nc.sync.dma_start(
                    (hist1 if l == 0 else hist2)[:, :, 0:BL],
                    d_h0T[l].rearrange("p (k b) -> p k b", b=BL),
                )
                ct = cpool.tile([128, KT * BL], F32, tag=f"c{l}")
                nc.sync.dma_start(ct[:], d_c0T[l])
                cprev.append(ct)

            rctx = ctx.enter_context(ExitStack())
            ps_l = [
                rctx.enter_context(
                    tc.tile_pool(name=f"ps_l{l}", bufs=3, space="PSUM")
                )
                for l in range(L)
            ]
            ps_g = rctx.enter_context(tc.tile_pool(name="ps_g1", bufs=2, space="PSUM"))

            def lstm_step(l, t):
                hist = hist1 if l == 0 else hist2
                gsrc = g0t if l == 0 else g1t
                ps = ps_l[l].tile([128, 16 * BL], F32, tag=f"psr{l}")
                first = True
                for m in range(MT):
                    for k in range(KT):
                        nc.tensor.matmul(
                            ps[:, ds(BL * _quad(m), BL)],
                            whh[l][:, k, ts(m, 128)],
                            hist[:, k, ds(BL * t, BL)],
                            start=first,
                            stop=(m == MT - 1 and k == KT - 1),
                        )
                        first = False
                sb = ewp.tile([128, 16 * BL], F32, tag=f"sb{l}")
                nc.vector.tensor_tensor(
                    sb[:].rearrange("p (q b) -> p q b", b=BL),
                    ps[:].rearrange("p (q b) -> p q b", b=BL),
                    gsrc[:, :, ds(BL * t, BL)],
                    op=ALU.add,
                )
                ga = ewp.tile([128, 16 * BL], F32, tag=f"ga{l}")
                nc.scalar.activation(ga[:, 0 : 12 * BL], sb[:, 0 : 12 * BL], AF.Sigmoid)
                nc.scalar.activation(
                    ga[:, 12 * BL : 16 * BL], sb[:, 12 * BL : 16 * BL], AF.Tanh
                )
                nch = KT * BL  # 16 = one [128, k, b] channel block
                ig = ewp.tile([128, nch], F32, tag=f"ig{l}")
                nc.vector.tensor_tensor(
                    ig[:], ga[:, 12 * BL : 16 * BL], ga[:, 0:nch], op=ALU.mult
                )
                fc = ewp.tile([128, nch], F32, tag=f"fc{l}")
                nc.vector.tensor_tensor(
                    fc[:], ga[:, nch : 2 * nch], cprev[l][:], op=ALU.mult
                )
                cn = cpool.tile([128, nch], F32, tag=f"c{l}")
                nc.vector.tensor_tensor(cn[:], ig[:], fc[:], op=ALU.add)
                tcn = ewp.tile([128, nch], F32, tag=f"tc{l}")
                nc.scalar.activation(tcn[:], cn[:], AF.Tanh)
                nc.vector.tensor_tensor(
                    hist[:, :, ds(BL * (t + 1), BL)],
                    ga[:, 2 * nch : 3 * nch].rearrange("p (k b) -> p k b", b=BL),
                    tcn[:].rearrange("p (k b) -> p k b", b=BL),
                    op=ALU.mult,
                )
                cprev[l] = cn

            # ---- interleaved recurrences: layer 2 lags by LAG steps ----
            for tt in range(T + LAG):
                if tt < T:
                    lstm_step(0, tt)
                if tt < T and tt % LAG == LAG - 1:
                    t0 = tt - LAG + 1
                    for m in range(MT):
                        ps = ps_g.tile([128, BL * LAG], F32, tag="psg1")
                        for k in range(KT):
                            nc.tensor.matmul(
                                ps[:],
                                wih[1][:, k, ts(m, 128)],
                                hist1[:, k, ds(BL * (t0 + 1), BL * LAG)],
                                start=(k == 0),
                                stop=(k == KT - 1),
                            )
                        nc.scalar.activation(
                            g1t[:, _quad(m), ds(BL * t0, BL * LAG)],
                            ps[:],
                            AF.Identity,
                            bias=gbias[:, 1, m : m + 1],
                        )
                if tt >= LAG:
                    lstm_step(1, tt - LAG)

            if debug:
                nc.gpsimd.dma_start(d_dbg_g0[:], g0t[:])
                nc.gpsimd.dma_start(d_dbg_h1[:], hist1[:])

            # ---- final h/c state outputs ----
            for l in range(L):
                hist = hist1 if l == 0 else hist2
                nc.gpsimd.dma_start(d_hout[l], hist[:, :, ds(BL * T, BL)])
                nc.sync.dma_start(d_cout[l], cprev[l][:])
            rctx.close()

            # ================= attention (batched over t) =================
            wainT = wpool.tile([128, KT, H], F16, tag="wainT")
            nc.sync.dma_start(wainT[:], d_wainT.rearrange("(k p) m -> p k m", p=128))
            waoutT = wpool.tile([128, 2 * KT, H], F16, tag="waoutT")
            nc.sync.dma_start(
                waoutT[:], d_waoutT.rearrange("(k p) m -> p k m", p=128)
            )
            ctx_sb = wpool.tile([S, BL, H], F16, tag="ctx")
            nc.sync.dma_start(ctx_sb[:], d_ctx.rearrange("b s h -> s b h"))
            ctxT_sb = wpool.tile([128, KT, BL, S], F16, tag="ctxT")
            for b in range(BL):
                nc.sync.dma_start(
                    ctxT_sb[:, :, b, :],
                    d_ctxT[b].rearrange("(k p) s -> p k s", p=128),
                )

            actx = ctx.enter_context(ExitStack())
            ps_q = actx.enter_context(tc.tile_pool(name="ps_q", bufs=2, space="PSUM"))
            ps_tr = actx.enter_context(tc.tile_pool(name="ps_at", bufs=2, space="PSUM"))

            # qT = W_attn_in @ h2T   [H, R]
            qT = embp.tile([128, KT, R], F16, tag="qT")
            for m in range(KT):
                ps = ps_q.tile([128, R], F32, tag="psq")
                for k in range(KT):
                    nc.tensor.matmul(
                        ps[:],
                        wainT[:, k, ts(m, 128)],
                        hist2[:, k, ds(BL, R)],
                        start=(k == 0),
                        stop=(k == KT - 1),
                    )
                nc.vector.tensor_copy(qT[:, m, :], ps[:])

            # per-batch: scores -> softmax -> attnT -> ctx^T
            ctxcat = embp.tile([128, KT, R], F16, tag="ctxcat")
            q4 = qT[:].rearrange("p k (t b) -> p k t b", b=BL)
            cc4 = ctxcat[:].rearrange("p k (t b) -> p k t b", b=BL)
            for b in range(BL):
                ps_sc = ps_tr.tile([T, S], F32, tag="ps_sc")
                for k in range(KT):
                    nc.tensor.matmul(
                        ps_sc[:],
                        q4[:, k, :, b],
                        ctxT_sb[:, k, b, :],
                        start=(k == 0),
                        stop=(k == KT - 1),
                    )
                exps = ewp.tile([T, S], F16, tag="exps")
                sume = ewp.tile([T, 1], F32, tag="sume")
                nc.scalar.activation(exps[:], ps_sc[:], AF.Exp, accum_out=sume[:])
                rsum = ewp.tile([T, 1], F32, tag="rsum")
                nc.vector.reciprocal(rsum[:], sume[:])
                attnw = ewp.tile([T, S], F16, tag="attnw")
                nc.vector.tensor_scalar_mul(attnw[:], exps[:], rsum[:])
                # attn_last output (t = T-1)
                nc.gpsimd.dma_start(d_attn[b : b + 1, :], attnw[T - 1 : T, :])
                # transpose attn -> [S, T]
                ps_at = ps_tr.tile([S, T], F16, tag="ps_at")
                nc.tensor.transpose(ps_at[:], attnw[:], ident[0:T, 0:T])
                attnT = ewp.tile([S, T], F16, tag="attnT")
                nc.vector.tensor_copy(attnT[:], ps_at[:])
                # ctx^T[b] = context_b^T @ attnT  -> scatter into ctxcat cols b::BL
                for m in range(KT):
                    ps_cx = ps_tr.tile([128, T], F32, tag="ps_cx")
                    nc.tensor.matmul(
                        ps_cx[:],
                        ctx_sb[:, b, ts(m, 128)],
                        attnT[:],
                        start=True,
                        stop=True,
                    )
                    nc.vector.tensor_copy(cc4[:, m, :, b], ps_cx[:])

            # outsT = tanh(W_attn_out @ [ctx; h2]^T)
            for m in range(KT):
                ps = ps_q.tile([128, R], F32, tag="psq")
                for k in range(2 * KT):
                    rhs = (
                        ctxcat[:, k, :]
                        if k < KT
                        else hist2[:, k - KT, ds(BL, R)]
                    )
                    nc.tensor.matmul(
                        ps[:],
                        waoutT[:, k, ts(m, 128)],
                        rhs,
                        start=(k == 0),
                        stop=(k == 2 * KT - 1),
                    )
                nc.scalar.activation(outsT[:, m, :], ps[:], AF.Tanh)

        # ================= projection + log-softmax =================
        with ExitStack() as ctx:
            wsp = ctx.enter_context(tc.tile_pool(name="wstream", bufs=2))
            brp = ctx.enter_context(tc.tile_pool(name="brep", bufs=1))
            lgp = ctx.enter_context(tc.tile_pool(name="lgts", bufs=1))
            accp = ctx.enter_context(tc.tile_pool(name="acc", bufs=1))
            outp = ctx.enter_context(tc.tile_pool(name="outp", bufs=2))
            ps_p = ctx.enter_context(tc.tile_pool(name="ps_p", bufs=2, space="PSUM"))

            logits_sb = lgp.tile([128, 2, V], F16)
            b_rep = brp.tile([128, V], F8)
            nc.sync.dma_start(b_rep[:], d_blin[:])
            sacc = accp.tile([128, 64], F32)
            wlin4 = d_wlinT.rearrange("(k p) v -> p k v", p=128)

            sups = []
            v0 = 0
            while v0 < V:
                sups.append((v0, min(VSUP, V - v0)))
                v0 += VSUP

            for si, (v0, vlen) in enumerate(sups):
                wt = wsp.tile([128, KT, VSUP], F8, tag="wt")
                nc.sync.dma_start(wt[:, :, 0:vlen], wlin4[:, :, ds(v0, vlen)])
                for m in range(2):
                    ps = ps_p.tile([128, VSUP], F32, tag="psp")
                    for j0 in range(0, vlen, VSUB):
                        jl = min(VSUB, vlen - j0)
                        for k in range(KT):
                            nc.tensor.matmul(
                                ps[:, ds(j0, jl)],
                                outsT[:, k, ts(m, 128)],
                                wt[:, k, ds(j0, jl)],
                                start=(k == 0),
                                stop=(k == KT - 1),
                            )
                    # logits = psum + b  (fp16, keeps bias out of the PE)
                    nc.vector.tensor_tensor(
                        logits_sb[:, m, ds(v0, vlen)],
                        ps[:, 0:vlen],
                        b_rep[:, ds(v0, vlen)],
                        op=ALU.add,
                    )
                    # sum of exp via ACT accumulate (exp values discarded)
                    scr = outp.tile([128, VSUP], F16, tag="scr")
                    nc.scalar.activation(
                        scr[:, 0:vlen],
                        logits_sb[:, m, ds(v0, vlen)],
                        AF.Exp,
                        accum_out=sacc[:, 32 * m + si : 32 * m + si + 1],
                    )

            nlz = accp.tile([128, 2], F32)
            for m in range(2):
                tot = accp.tile([128, 1], F32, tag="tot")
                nc.vector.reduce_sum(
                    tot[:], sacc[:, ds(32 * m, len(sups))], axis=mybir.AxisListType.X
                )
                lnz = accp.tile([128, 1], F32, tag="lnz")
                nc.scalar.activation(lnz[:], tot[:], AF.Ln)
                nc.vector.tensor_scalar_mul(nlz[:, m : m + 1], lnz[:], -1.0)

            for si, (v0, vlen) in enumerate(sups):
                for m in range(2):
                    ot = outp.tile([128, VSUP], F16, tag="ot")
                    if (si + m) % 2 == 0:
                        nc.vector.tensor_scalar_add(
                            ot[:, 0:vlen],
                            logits_sb[:, m, ds(v0, vlen)],
                            nlz[:, m : m + 1],
                        )
                    else:
                        nc.scalar.activation(
                            ot[:, 0:vlen],
                            logits_sb[:, m, ds(v0, vlen)],
                            AF.Identity,
                            bias=nlz[:, m : m + 1],
                        )
                    nc.sync.dma_start(
                        d_logp[ds(128 * m, 128), ds(v0, vlen)], ot[:, 0:vlen]
                    )

    nc.compile()
    return nc


_NC_CACHE = {}


def _get_nc():
    if "nc" not in _NC_CACHE:
        _NC_CACHE["nc"] = build_program()
    return _NC_CACHE["nc"]


def kernel(
    tokens,
    h0,
    c0,
    context,
    embedding,
    Wih,
    Whh,
    bih,
    bhh,
    W_attn_in,
    W_attn_out,
    W_lin,
    b_lin,
):
    tokens = np.asarray(tokens)
    h0 = np.asarray(h0, dtype=np.float32)
    c0 = np.asarray(c0, dtype=np.float32)
    context = np.asarray(context, dtype=np.float32)
    embedding = np.asarray(embedding, dtype=np.float32)
    Wih = np.asarray(Wih, dtype=np.float32)
    Whh = np.asarray(Whh, dtype=np.float32)
    bih = np.asarray(bih, dtype=np.float32)
    bhh = np.asarray(bhh, dtype=np.float32)
    W_attn_in = np.asarray(W_attn_in, dtype=np.float32)
    W_attn_out = np.asarray(W_attn_out, dtype=np.float32)
    W_lin = np.asarray(W_lin, dtype=np.float32)
    b_lin = np.asarray(b_lin, dtype=np.float32)

    f16 = np.float16
    emb16 = embedding.astype(f16)
    WhhT = np.ascontiguousarray(np.transpose(Whh, (0, 2, 1)).astype(f16))
    WihT = np.ascontiguousarray(np.transpose(Wih, (0, 2, 1)).astype(f16))
    gbias = np.ascontiguousarray(
        (bih + bhh).astype(np.float32).reshape(L, MT, 128).transpose(0, 2, 1)
    )
    import ml_dtypes

    WaInT = np.ascontiguousarray(W_attn_in.T.astype(f16))
    WaOutT = np.ascontiguousarray(W_attn_out.T.astype(f16))
    WlinT = np.ascontiguousarray(W_lin.T.astype(ml_dtypes.float8_e4m3))
    blin = np.ascontiguousarray(
        np.broadcast_to(
            b_lin.astype(ml_dtypes.float8_e4m3).reshape(1, V), (128, V)
        )
    )

    def pack_T(x):  # [BL, 512] -> [128, KT*BL] transposed k-major packing
        return np.ascontiguousarray(
            x.T.reshape(KT, 128, BL).transpose(1, 0, 2).reshape(128, KT * BL)
        )

    in_maps = []
    for core in range(NCORES):
        bs = slice(BL * core, BL * (core + 1))
        tok = tokens[bs, :].astype(np.int64)
        tokflat = tok.T.reshape(-1)  # rows r = t*BL + b
        embT_in = np.ascontiguousarray(
            emb16[tokflat].reshape(R, KT, 128).transpose(2, 1, 0)
        )
        h0T = np.stack([pack_T(h0[l, bs, :]) for l in range(L)]).astype(f16)
        c0T = np.stack([pack_T(c0[l, bs, :]) for l in range(L)]).astype(np.float32)
        ctxl = np.ascontiguousarray(context[bs].astype(f16))
        ctxTl = np.ascontiguousarray(np.transpose(context[bs], (0, 2, 1)).astype(f16))
        in_maps.append(
            dict(
                embT_in=embT_in,
                whhT=WhhT,
                wihT=WihT,
                gbias=gbias,
                h0T=h0T,
                c0T=c0T,
                wattn_inT=WaInT,
                wattn_outT=WaOutT,
                ctx=ctxl,
                ctxT=ctxTl,
                wlinT=WlinT,
                blin=blin,
            )
        )

    nc = _get_nc()
    trace = os.environ.get("KERNEL_TRACE", "0") == "1"
    res = run_bass_kernel_spmd(
        nc, in_maps, core_ids=list(range(NCORES)), trace=trace
    )
    _NC_CACHE["last_result"] = res
    outs = res.results

    log_probs = np.empty((T, B, V), np.float32)
    hT = np.empty((L, B, H), np.float32)
    cT = np.empty((L, B, H), np.float32)
    attn_last = np.empty((B, S), np.float32)

    def unpack_T(x):  # [128, KT*BL] -> [BL, 512]
        return (
            x.reshape(128, KT, BL).transpose(2, 1, 0).reshape(BL, H)
        )

    for core in range(NCORES):
        o = outs[core]
        bs = slice(BL * core, BL * (core + 1))
        log_probs[:, bs, :] = o["logp"].astype(np.float32).reshape(T, BL, V)
        for l in range(L):
            hT[l, bs, :] = unpack_T(o["hT_out"][l])
            cT[l, bs, :] = unpack_T(o["cT_out"][l])
        attn_last[bs, :] = o["attn_out"]

    return log_probs, hT, cT, attn_last
